# revision 25
# baseline (speedup 1.0000x reference)
"""Based-style linear attention (Taylor feature map) on 8 Trainium2 cores.

Math: reference computes, per head h (FDIM=16, HEAD_DIM=64):
    q,k = HS@Wq, HS@Wk    (per-head 16 dims), v = HS@Wv (per-head 64 dims)
    phi(x) = [1, x/2, outer(x,x)/(sqrt(2)*4)]      (273 dims)
    y_t = sum_{s<=t} (phi(q_t).phi(k_s)) v_s / sum_{s<=t} phi(q_t).phi(k_s)
    out = concat_h(y) @ Wo

Key identity: phi(q).phi(k) = 1 + S/4 + S^2/32 where S = q.k (16-dim dot)
            = Square(S/sqrt(32) + 1/sqrt(2)) + 1/2.
So scores come from 16-dim dot products + one ACT Square pass; the 273-dim
feature map is never materialized.

Sharding: head-parallel, no collectives. 16 virtual heads (12 real + 4
zero dummies), 2 per core. Each core: q/k/v projections for its heads
(full L=1024), causal chunked attention (8 chunks of 128), partial
output y_heads @ Wo_rows. Host sums the 8 partial outputs.

Layout is chosen so every matmul has a large free dim (N up to 512):
 - scores: per kv-chunk j, one matmul produces S^T[kv=128, t=j*128..1024]
   for all later query positions at once (k as stationary operand).
 - A@V: V-as-stationary, rhs = Square(S^T) big tile -> accumulates
   num^T[vc=65, t=0..1024] across j. No transposes anywhere.
 - the "+1/2" in every causal score is folded in via constant-matrix
   matmuls (htri for the diagonal chunk, sel x colsum for prior chunks);
   only diagonal 128x128 blocks need an elementwise tri-mask.
 - den rides along as v-column 64 (ones), y = num * (1/den) with the
   reciprocal row broadcast across partitions by a K=1 matmul.

Matmul operands are bf16 (PE streams 2 B/lane/cycle, so bf16 is 2x fp32;
products of bf16 pairs accumulate exactly in fp32 PSUM). PSUM, den and
the reciprocal stay fp32.
"""

import math

import numpy as np
import ml_dtypes

import concourse.bass as bass
import concourse.mybir as mybir
import concourse.tile as tile
from concourse import bacc
from concourse.bass_utils import run_bass_kernel_spmd

L = 1024
D = 768
H = 12
FD = 16
HD = 64
NCORE = 8
NCH = 8  # L chunks of 128
KB = 6  # contraction blocks of 128 over D
F32 = mybir.dt.float32
BF16 = mybir.dt.bfloat16

# dtype knobs (bf16 = 2x PE throughput; flip to F32 to trade speed for bits)
DT_PROJ = BF16
DT_ATT = BF16
DT_OUT = BF16

A_SCALE = 1.0 / math.sqrt(32.0)
A_BIAS = 1.0 / math.sqrt(2.0)

_compiled_nc = None
_last_in_maps = None


def _np_dt(dt):
    return ml_dtypes.bfloat16 if dt == BF16 else np.float32


def _bank_splits(lo, hi, bank=512):
    """Split [lo, hi) at multiples of `bank` (PSUM bank boundaries)."""
    out = []
    a = lo
    while a < hi:
        b = min(hi, (a // bank + 1) * bank)
        out.append((a, b))
        a = b
    return out


def _build_nc():
    nc = bacc.Bacc("TRN2", target_bir_lowering=False, debug=False, num_devices=NCORE)

    hsT = nc.dram_tensor("hsT", [D, L], DT_PROJ, kind="ExternalInput")
    wqv = nc.dram_tensor("wqv", [D, 258], DT_PROJ, kind="ExternalInput")
    wo = nc.dram_tensor("wo", [128, D], DT_OUT, kind="ExternalInput")
    # consts packed: tri 0:128 | htri 128:256 | ones8 256:320 | sel 320:1344
    c_all = nc.dram_tensor("c_all", [128, 1344], DT_ATT, kind="ExternalInput")
    out = nc.dram_tensor("out", [L, D], DT_OUT, kind="ExternalOutput")

    with tile.TileContext(nc) as tc:
        with (
            tc.tile_pool(name="cst", bufs=1) as cst,
            tc.tile_pool(name="sqp", bufs=4) as sqp,
            tc.tile_pool(name="wrk", bufs=2) as wrk,
        ):
            # ---- load inputs to SBUF, ordered so the first projection
            # matmul (needs wqv kb=0 + hs kb=0) can start asap ----
            wqv_re = wqv.ap().rearrange("(po pi) f -> pi po f", pi=128)
            hs_re = hsT.ap().rearrange("(po pi) f -> pi po f", pi=128)
            wqv_sb = cst.tile([128, KB, 258], DT_PROJ, tag="wqv")
            hs_sb = [
                cst.tile([128, L], DT_PROJ, tag=f"hs{kb}", name=f"hs{kb}")
                for kb in range(KB)
            ]
            for kb in range(KB):
                nc.sync.dma_start(out=wqv_sb[:, kb, :], in_=wqv_re[:, kb, :])
                nc.sync.dma_start(out=hs_sb[kb], in_=hs_re[:, kb, :])
            wk_sb = wqv_sb[:, :, 0:64]
            wq_sb = wqv_sb[:, :, 64:128]
            wv_sb = wqv_sb[:, :, 128:258]
            call_sb = cst.tile([128, 1344], DT_ATT, tag="call")
            nc.sync.dma_start(out=call_sb, in_=c_all.ap())
            tri_sb = call_sb[:, 0:128]
            htri_sb = call_sb[:, 128:256]
            ones8_sb = call_sb[:, 256:320]
            sel_sb = call_sb[0:8, 320:1344]
            # wo split into per-head tiles so o-proj operands share base 0
            wo_sb = []
            for h in range(2):
                t = cst.tile([64, D], DT_OUT, tag=f"wo{h}", name=f"wo{h}")
                nc.sync.dma_start(out=t, in_=wo.ap()[64 * h : 64 * h + 64, :])
                wo_sb.append(t)
            bias_sb = cst.tile([128, 1], F32, tag="bias")
            nc.vector.memset(bias_sb, A_BIAS)
            # row of ones at partition 64, for the den-reciprocal broadcast
            ones64_sb = cst.tile([65, 64], F32, tag="ones64")
            nc.vector.memset(ones64_sb, 0.0)
            nc.vector.memset(ones64_sb[64:65, :], 1.0)

            kq_sb = cst.tile([64, 2048], DT_ATT, tag="kq")
            vx_sb = cst.tile([128, NCH, 130], DT_ATT, tag="vx")
            colsum_sb = cst.tile([8, 130], DT_ATT, tag="colsum")

            # ================= projections =================
            with tc.tile_pool(name="ps1", bufs=3, space="PSUM") as ps1:
                # q/k -> kq_sb [64, 2048]; partitions 0-15 head0, 32-47 head1
                # (rest zero); cols 0-1023 = k^T, 1024-2047 = q^T
                for w_sb, coff in ((wk_sb, 0), (wq_sb, 1024)):
                    for half in range(2):
                        p = ps1.tile([64, 512], F32, tag="pB", name=f"pqk{coff}_{half}")
                        for kb in range(KB):
                            nc.tensor.matmul(
                                p,
                                w_sb[:, kb, :],
                                hs_sb[kb][:, half * 512 : (half + 1) * 512],
                                start=(kb == 0),
                                stop=(kb == KB - 1),
                            )
                        nc.vector.tensor_copy(
                            kq_sb[:, coff + half * 512 : coff + (half + 1) * 512], p
                        )
                # v -> vx_sb [128, 8, 130]: cols 0-63 v_h0, 64 ones,
                # 65-128 v_h1, 129 ones
                for ch in range(NCH):
                    pv = ps1.tile([128, 130], F32, tag="pB", name=f"pv{ch}")
                    for kb in range(KB):
                        nc.tensor.matmul(
                            pv,
                            hs_sb[kb][:, ch * 128 : (ch + 1) * 128],
                            wv_sb[:, kb, :],
                            start=(kb == 0),
                            stop=(kb == KB - 1),
                        )
                    nc.vector.tensor_copy(vx_sb[:, ch, :], pv)
                nc.vector.memset(vx_sb[:, :, 64], 1.0)
                nc.vector.memset(vx_sb[:, :, 129], 1.0)

                # per-chunk column sums of vx (inter-chunk +1/2 term)
                pcs = ps1.tile([8, 130], F32, tag="pB", name="pcs")
                for ch in range(NCH):
                    nc.tensor.matmul(
                        pcs,
                        ones8_sb[:, ch * 8 : (ch + 1) * 8],
                        vx_sb[:, ch, :],
                        start=(ch == 0),
                        stop=(ch == NCH - 1),
                    )
                nc.vector.tensor_copy(colsum_sb, pcs)

            # ================= attention =================
            yT_sb = [
                cst.tile([64, L], DT_OUT, tag=f"yT{h}", name=f"yT{h}") for h in range(2)
            ]
            with tc.tile_pool(name="psnum", bufs=1, space="PSUM") as psnum:
                nums = [
                    psnum.tile([65, L], F32, tag=f"pN{h}", name=f"num{h}")
                    for h in range(2)
                ]
                with tc.tile_pool(name="psa", bufs=2, space="PSUM") as psa:
                    for j in range(NCH):
                        tlo = j * 128
                        width = L - tlo
                        for h in range(2):
                            pa = psa.tile([128, 1024], F32, tag="pA", name=f"pa{j}_{h}")[
                                :, :width
                            ]
                            for a, b in _bank_splits(0, width):
                                nc.tensor.matmul(
                                    pa[:, a:b],
                                    kq_sb[32 * h : 32 * h + 32, tlo : tlo + 128],
                                    kq_sb[
                                        32 * h : 32 * h + 32,
                                        1024 + tlo + a : 1024 + tlo + b,
                                    ],
                                    start=True,
                                    stop=True,
                                )
                            sq = sqp.tile([128, 1024], DT_ATT, tag="sq", name=f"sq{j}_{h}")[
                                :, :width
                            ]
                            nc.scalar.activation(
                                out=sq,
                                in_=pa,
                                func=mybir.ActivationFunctionType.Square,
                                scale=A_SCALE,
                                bias=bias_sb,
                            )
                            # mask the diagonal block (i == j)
                            nc.vector.tensor_mul(sq[:, 0:128], sq[:, 0:128], tri_sb)
                            # num^T += V_j^T-stationary @ sq
                            for a, b in _bank_splits(tlo, L):
                                nc.tensor.matmul(
                                    nums[h][:, a:b],
                                    vx_sb[:, j, 65 * h : 65 * h + 65],
                                    sq[:, a - tlo : b - tlo],
                                    start=(j == 0),
                                    stop=False,
                                )
                    # intra-chunk +1/2 term: 0.5 * prefix-sums of V_i
                    for h in range(2):
                        for i in range(NCH):
                            nc.tensor.matmul(
                                nums[h][:, i * 128 : (i + 1) * 128],
                                vx_sb[:, i, 65 * h : 65 * h + 65],
                                htri_sb,
                                start=False,
                                stop=False,
                            )
                        # inter-chunk +1/2 term: 0.5 * sum of prior colsums
                        for a, b in _bank_splits(0, L):
                            nc.tensor.matmul(
                                nums[h][:, a:b],
                                colsum_sb[:, 65 * h : 65 * h + 65],
                                sel_sb[:, a:b],
                                start=False,
                                stop=True,
                            )

                # y^T = num^T[0:64] / den  (den = row 64)
                with tc.tile_pool(name="ps2", bufs=2, space="PSUM") as ps2:
                    for h in range(2):
                        # custom-DVE ops require base partition 0: run the
                        # approx reciprocal over the whole tile (rows 0-63
                        # are garbage, only den row 64 is used)
                        rc = wrk.tile([65, L], F32, tag="rc")
                        nc.vector.reciprocal_approx_fast(out=rc, in_=nums[h])
                        prb = ps2.tile([64, L], F32, tag="prb", name=f"prb{h}")
                        for a, b in _bank_splits(0, L):
                            nc.tensor.matmul(
                                prb[:, a:b],
                                ones64_sb[64:65, :],
                                rc[64:65, a:b],
                                start=True,
                                stop=True,
                            )
                        rb = wrk.tile([64, L], F32, tag="rb")
                        nc.any.tensor_copy(rb, prb)
                        nc.vector.tensor_mul(yT_sb[h], nums[h][0:64, :], rb)

            # ================= output projection =================
            with tc.tile_pool(name="ps3", bufs=4, space="PSUM") as ps3:
                for i in range(NCH):
                    osb = wrk.tile([128, D], DT_OUT, tag="osb")
                    for a, b in ((0, 512), (512, 768)):
                        po = ps3.tile([128, 512], F32, tag="po", name=f"po{i}_{a}")[
                            :, : b - a
                        ]
                        for h in range(2):
                            nc.tensor.matmul(
                                po,
                                yT_sb[h][:, i * 128 : (i + 1) * 128],
                                wo_sb[h][:, a:b],
                                start=(h == 0),
                                stop=(h == 1),
                            )
                        nc.any.tensor_copy(osb[:, a:b], po)
                    nc.sync.dma_start(out=out.ap()[i * 128 : (i + 1) * 128, :], in_=osb)

    nc.finalize()
    return nc


def _host_consts():
    s = np.arange(128)[:, None]
    t = np.arange(128)[None, :]
    tri = (s <= t).astype(np.float32)
    htri = 0.5 * tri
    sel = np.zeros((8, 1024), dtype=np.float32)
    for i in range(8):
        sel[:i, i * 128 : (i + 1) * 128] = 0.5
    ones8 = np.zeros((128, 64), dtype=np.float32)
    for ch in range(8):
        ones8[:, ch * 8 + ch] = 1.0
    return tri, htri, sel, ones8


def kernel(hidden_states, Wq, Wk, Wv, Wo):
    global _compiled_nc, _last_in_maps
    hs = np.asarray(hidden_states, dtype=np.float32)[0]  # [L, D]
    Wq = np.asarray(Wq, dtype=np.float32)
    Wk = np.asarray(Wk, dtype=np.float32)
    Wv = np.asarray(Wv, dtype=np.float32)
    Wo = np.asarray(Wo, dtype=np.float32)

    if _compiled_nc is None:
        _compiled_nc = _build_nc()
    nc = _compiled_nc

    proj_dt = _np_dt(DT_PROJ)
    att_dt = _np_dt(DT_ATT)
    out_dt = _np_dt(DT_OUT)

    hsT = np.ascontiguousarray(hs.T).astype(proj_dt)  # [D, L]
    tri, htri, sel, ones8 = _host_consts()
    c_all = np.zeros((128, 1344), dtype=np.float32)
    c_all[:, 0:128] = tri
    c_all[:, 128:256] = htri
    c_all[:, 256:320] = ones8
    c_all[0:8, 320:1344] = sel
    c_all = c_all.astype(att_dt)

    in_maps = []
    for c in range(NCORE):
        heads = [2 * c, 2 * c + 1]
        wk_c = np.zeros((D, 64), dtype=np.float32)
        wq_c = np.zeros((D, 64), dtype=np.float32)
        wv_c = np.zeros((D, 130), dtype=np.float32)
        wo_c = np.zeros((128, D), dtype=np.float32)
        for hi, h in enumerate(heads):
            if h >= H:
                continue
            wk_c[:, 32 * hi : 32 * hi + FD] = Wk[:, h * FD : (h + 1) * FD]
            wq_c[:, 32 * hi : 32 * hi + FD] = Wq[:, h * FD : (h + 1) * FD]
            wv_c[:, 65 * hi : 65 * hi + HD] = Wv[:, h * HD : (h + 1) * HD]
            wo_c[64 * hi : 64 * hi + HD, :] = Wo[h * HD : (h + 1) * HD, :]
        wqv_c = np.concatenate([wk_c, wq_c, wv_c], axis=1)
        in_maps.append(
            {
                "hsT": hsT,
                "wqv": wqv_c.astype(proj_dt),
                "wo": wo_c.astype(out_dt),
                "c_all": c_all,
            }
        )

    _last_in_maps = in_maps
    res = run_bass_kernel_spmd(nc, in_maps, list(range(NCORE)))
    acc = np.zeros((L, D), dtype=np.float32)
    for c in range(NCORE):
        acc += np.asarray(res.results[c]["out"], dtype=np.float32)
    return acc.reshape(1, L, D)
